# revision 45
# baseline (speedup 1.0000x reference)
"""KVGather kernel for Trainium2 (8 NeuronCores, SPMD data-parallel over batch).

Problem: kv (16, 64, 196, 128) f32; r_idx/r_weight (16, 64, 4).
out[n, p, t] = r_weight[n, p, t] * kv[n, r_idx[n, p, t]]  -> (16, 64, 4, 196, 128)

Strategy (per core: 2 batches). The kernel is DMA-bound (output is 4x the
input), so both sides of the HBM traffic run in bf16:

  - kv is sent to the device as bf16 (one term). The gather itself is exact
    (one-hot matmul accumulates in fp32 PSUM), so the only input error is the
    bf16 rounding of kv (~2^-9 relative).
  - Gather on the PE array as a one-hot matmul: psum[m, :] = sel_mh.T @
    rhs_chunk, where sel is a host-built {0,1} bf16 selection matrix and rhs
    holds the batch's kv regions flat-packed across 128 partitions
    (partition h*64 + r = half h of region r).
  - PSUM eviction fuses the r_weight multiply (per-partition f32 scale) and
    the f32->bf16 down-convert, alternating DVE/ACT (GPSIMD has no PSUM
    port). Eviction tiles span 2 PSUM banks ([128, 1024] f32) to amortize
    per-instruction overhead; PSUM = 4 such slots = all 8 banks.
  - Output DMAs are [128 x 1024] bf16 with 2 KB contiguous runs per
    partition, alternating between the two HWDGE rings (SP issues the
    DVE-evicted tiles, ACT its own) so neither ring's FIFO head-of-line
    blocks the store stream. Host converts bf16 -> f32 on assembly.
  - All input loads issue up front, with both batches' stripes alternating
    across both HWDGE rings so each ring's store stream is frozen behind
    input for only ~half the input time; 24 out-tile slots give the
    production pipeline enough elasticity to absorb that freeze without
    stalling (rings are FIFO, so stores cannot overtake input).
  - 8 junk matmuls on sel data pre-ramp the PE pstate while the real
    matmuls are still blocked on input-DMA completion semaphores (free).

Total HBM traffic per core: 6.7 MB in + 25.7 MB out = ~32.4 MB, which at the
~358 GB/s per-core HBM ceiling (16 SDMA engines x ~22.4 GB/s) is ~90 us of
DMA busy; measured exec ~102 us = ~7 us framework preamble + ~91 us DMA span
at ~96% engine duty + ~4 us drain tail. The 16 engines each run at their
per-engine HBM share, so this is the traffic floor for bf16 output.

Everything is static: one compiled program for all cores and all inputs;
indices/weights only enter through input tensors (sel, wt).
"""

import sys

if "/opt/trn_rl_repo" not in sys.path:
    sys.path.insert(0, "/opt/trn_rl_repo")

import numpy as np
import ml_dtypes

import concourse.bass as bass
import concourse.bacc as bacc
import concourse.mybir as mybir
from concourse import tile
from concourse.bass_utils import run_bass_kernel_spmd

BF16 = ml_dtypes.bfloat16

# Problem constants
N, P2, TOPK, W2, C_KV = 16, 64, 4, 196, 128
REG = W2 * C_KV  # 25088 f32 per region
RHALF = REG // 2  # 12544 per region half
N_CORES = 8
B = N // N_CORES  # batches per core = 2
G = P2 * TOPK  # gathers per batch = 256
MG = G // 128  # m-groups of 128 gathers = 2
CH = 512  # matmul chunk (one PSUM bank of f32)
OCH = 1024  # eviction/store tile (two PSUM banks; 2 KB bf16 per partition)

_COMPILED = None
RUN_KWARGS = {}  # test harness may set e.g. {"trace": True}
LAST_RESULTS = None  # BassKernelResults of the last run (for profiling)


def _build():
    nc = bacc.Bacc("TRN2", target_bir_lowering=False, debug=False, num_devices=N_CORES)
    f32, bf16 = mybir.dt.float32, mybir.dt.bfloat16

    kv_d = nc.dram_tensor("kvb", [B, 128, RHALF], bf16, kind="ExternalInput").ap()
    sel_d = nc.dram_tensor("sel", [128, B * MG * 2 * 128], bf16, kind="ExternalInput").ap()
    wt_d = nc.dram_tensor("wt", [128, B * MG], f32, kind="ExternalInput").ap()
    out_d = nc.dram_tensor("out", [B, G, REG], bf16, kind="ExternalOutput").ap()

    with tile.TileContext(nc) as tc:
        with (
            tc.tile_pool(name="rhs", bufs=2) as rhs_pool,
            tc.tile_pool(name="const", bufs=1) as const_pool,
            tc.tile_pool(name="psum", bufs=4, space="PSUM") as psum_pool,
            tc.tile_pool(name="outp", bufs=36) as out_pool,
        ):
            sel_sb = const_pool.tile([128, B * MG * 2 * 128], bf16)
            wt_sb = const_pool.tile([128, B * MG], f32)

            # All input loads issued up front: the batch-0 stripes gate the
            # first matmuls; batch-1 fills DMA-engine idle time during the
            # early compute ramp instead of queueing behind batch-0 stores.
            # The first (small) stripe goes ahead of even sel/wt so the first
            # matmul's data dependency clears as early as possible.
            stripes = [(0, 1024), (1024, 3584), (3584, 6656), (6656, 9728), (9728, RHALF)]
            kv_sbs = [rhs_pool.tile([128, RHALF], bf16, tag="kv", name=f"kv{b}") for b in range(B)]
            nc.sync.dma_start(sel_sb[:], sel_d)
            nc.sync.dma_start(wt_sb[:], wt_d)
            # All input up front: batch-0 stripes on the SP ring (no stores
            # exist yet to block), batch-1 stripes alternating across BOTH
            # HWDGE rings so each ring's store stream is frozen only ~half
            # the input time - the 24 out-slots absorb that much production.
            for sidx, (s0, s1) in enumerate(stripes):
                ring = nc.sync if sidx % 2 == 0 else nc.scalar
                ring.dma_start(kv_sbs[0][:, s0:s1], kv_d[0][:, s0:s1])
            for ci in range(8):
                c0_, c1_ = ci * 1568, (ci + 1) * 1568
                ring = nc.sync if ci % 2 == 0 else nc.scalar
                ring.dma_start(kv_sbs[1][:, c0_:c1_], kv_d[1][:, c0_:c1_])

            # PE pstate warmup: 8 junk matmuls on sel data. These are free —
            # the first real matmuls are blocked on input-DMA completion
            # semaphores until ~15 us anyway — and they pre-ramp the PE clock
            # so real matmuls start at ~377 ns/512-col instead of ~630 ns.
            warm_ps = psum_pool.tile([128, 1024], f32, tag="ps", name="warm_ps")
            for w in range(8):
                half = (w % 2) * 512
                nc.tensor.matmul(
                    warm_ps[:, half : half + 512],
                    sel_sb[:, 0:128],
                    sel_sb[:, 256 : 256 + 512],
                    start=True,
                    stop=True,
                )

            ev = 0
            for b in range(B):
                kv_sb = kv_sbs[b]
                for mg in range(MG):
                    wcol = wt_sb[:, b * MG + mg : b * MG + mg + 1]
                    for h in range(2):
                        si = (b * MG + mg) * 2 + h
                        sel_ap = sel_sb[:, si * 128 : (si + 1) * 128]
                        for o0 in range(0, RHALF, OCH):
                            ow = min(OCH, RHALF - o0)
                            ps = psum_pool.tile([128, ow], f32, tag="ps")
                            for c0 in range(0, ow, CH):
                                cw = min(CH, ow - c0)
                                nc.tensor.matmul(
                                    ps[:, c0 : c0 + cw],
                                    sel_ap,
                                    kv_sb[:, o0 + c0 : o0 + c0 + cw],
                                    start=True,
                                    stop=True,
                                )
                            ot = out_pool.tile([128, ow], bf16, tag="ot")
                            dst = out_d[
                                b,
                                mg * 128 : (mg + 1) * 128,
                                h * RHALF + o0 : h * RHALF + o0 + ow,
                            ]
                            # alternate DVE/ACT evictions; each tile's store is
                            # issued from the matching HWDGE ring (SP for DVE
                            # tiles, ACT for its own) to halve per-ring dispatch
                            # pressure
                            if ev % 2 == 0:
                                nc.vector.tensor_scalar_mul(ot[:], ps[:], wcol)
                                nc.sync.dma_start(dst, ot[:])
                            else:
                                nc.scalar.activation(
                                    ot[:],
                                    ps[:],
                                    mybir.ActivationFunctionType.Copy,
                                    scale=wcol,
                                )
                                nc.scalar.dma_start(dst, ot[:])
                            ev += 1

    nc.compile()
    return nc


def _get_nc():
    global _COMPILED
    if _COMPILED is None:
        _COMPILED = _build()
    return _COMPILED


def _prep_core(kv_c: np.ndarray, idx_c: np.ndarray, w_c: np.ndarray) -> dict:
    """kv_c (B, 64, 196, 128) f32, idx_c (B, 64, 4) int, w_c (B, 64, 4) f32."""
    # rhs layout [B, 128, RHALF]: partition h*64 + r = half h of region r (flat)
    kvr = (
        kv_c.reshape(B, P2, 2, RHALF).transpose(0, 2, 1, 3).reshape(B, 128, RHALF)
    )
    kvb = kvr.astype(BF16)

    idx_f = idx_c.reshape(B, G).astype(np.int64)
    w_f = w_c.reshape(B, G).astype(np.float32)

    sel = np.zeros((128, B, MG, 2, 128), dtype=BF16)
    k = np.arange(128)[:, None]
    for b in range(B):
        for mg in range(MG):
            im = idx_f[b, mg * 128 : (mg + 1) * 128][None, :]
            sel[:, b, mg, 0] = (k == im).astype(BF16)
            sel[:, b, mg, 1] = (k == im + 64).astype(BF16)
    sel = sel.reshape(128, B * MG * 2 * 128)

    wt = np.zeros((128, B * MG), dtype=np.float32)
    for b in range(B):
        for mg in range(MG):
            wt[:, b * MG + mg] = w_f[b, mg * 128 : (mg + 1) * 128]

    return {"kvb": kvb, "sel": sel, "wt": wt}


def kernel(r_idx: np.ndarray, r_weight: np.ndarray, kv: np.ndarray) -> np.ndarray:
    global LAST_RESULTS
    nc = _get_nc()
    kv = np.asarray(kv, dtype=np.float32)
    r_idx = np.asarray(r_idx)
    r_weight = np.asarray(r_weight, dtype=np.float32)

    in_maps = [
        _prep_core(
            kv[c * B : (c + 1) * B],
            r_idx[c * B : (c + 1) * B],
            r_weight[c * B : (c + 1) * B],
        )
        for c in range(N_CORES)
    ]

    res = run_bass_kernel_spmd(nc, in_maps, core_ids=list(range(N_CORES)), **RUN_KWARGS)
    LAST_RESULTS = res

    out = np.empty((N, P2, TOPK, W2, C_KV), dtype=np.float32)
    for c in range(N_CORES):
        o = np.asarray(res.results[c]["out"])  # (B, G, REG) bf16
        out[c * B : (c + 1) * B] = o.reshape(B, P2, TOPK, W2, C_KV).astype(np.float32)
    return out


# revision 47
# speedup vs baseline: 1.0134x; 1.0134x over previous
"""KVGather kernel for Trainium2 (8 NeuronCores, SPMD data-parallel over batch).

Problem: kv (16, 64, 196, 128) f32; r_idx/r_weight (16, 64, 4).
out[n, p, t] = r_weight[n, p, t] * kv[n, r_idx[n, p, t]]  -> (16, 64, 4, 196, 128)

Strategy (per core: 2 batches). The kernel is DMA-bound (output is 4x the
input), so both sides of the HBM traffic run in bf16:

  - kv is sent to the device as bf16 (one term). The gather itself is exact
    (one-hot matmul accumulates in fp32 PSUM), so the only input error is the
    bf16 rounding of kv (~2^-9 relative).
  - Gather on the PE array as a one-hot matmul: psum[m, :] = sel_mh.T @
    rhs_chunk, where sel is a host-built {0,1} bf16 selection matrix and rhs
    holds the batch's kv regions flat-packed across 128 partitions
    (partition h*64 + r = half h of region r).
  - PSUM eviction fuses the r_weight multiply (per-partition f32 scale) and
    the f32->bf16 down-convert, alternating DVE/ACT (GPSIMD has no PSUM
    port). Eviction tiles span 2 PSUM banks ([128, 1024] f32) to amortize
    per-instruction overhead; PSUM = 4 such slots = all 8 banks.
  - Output DMAs are [128 x 1024] bf16 with 2 KB contiguous runs per
    partition, alternating between the two HWDGE rings (SP issues the
    DVE-evicted tiles, ACT its own) so neither ring's FIFO head-of-line
    blocks the store stream. Host converts bf16 -> f32 on assembly.
  - All input loads issue up front, with both batches' stripes alternating
    across both HWDGE rings so each ring's store stream is frozen behind
    input for only ~half the input time; 24 out-tile slots give the
    production pipeline enough elasticity to absorb that freeze without
    stalling (rings are FIFO, so stores cannot overtake input).
  - 8 junk matmuls on sel data pre-ramp the PE pstate while the real
    matmuls are still blocked on input-DMA completion semaphores (free).

Total HBM traffic per core: 6.7 MB in + 25.7 MB out = ~32.4 MB, which at the
~358 GB/s per-core HBM ceiling (16 SDMA engines x ~22.4 GB/s) is ~90 us of
DMA busy; measured exec ~102 us = ~7 us framework preamble + ~91 us DMA span
at ~96% engine duty + ~4 us drain tail. The 16 engines each run at their
per-engine HBM share, so this is the traffic floor for bf16 output.

Everything is static: one compiled program for all cores and all inputs;
indices/weights only enter through input tensors (sel, wt).
"""

import sys

if "/opt/trn_rl_repo" not in sys.path:
    sys.path.insert(0, "/opt/trn_rl_repo")

import numpy as np
import ml_dtypes

import concourse.bass as bass
import concourse.bacc as bacc
import concourse.mybir as mybir
from concourse import tile
from concourse.bass_utils import run_bass_kernel_spmd

BF16 = ml_dtypes.bfloat16

# Problem constants
N, P2, TOPK, W2, C_KV = 16, 64, 4, 196, 128
REG = W2 * C_KV  # 25088 f32 per region
RHALF = REG // 2  # 12544 per region half
N_CORES = 8
B = N // N_CORES  # batches per core = 2
G = P2 * TOPK  # gathers per batch = 256
MG = G // 128  # m-groups of 128 gathers = 2
CH = 512  # matmul chunk (one PSUM bank of f32)
OCH = 1024  # eviction/store tile (two PSUM banks; 2 KB bf16 per partition)

_COMPILED = None
RUN_KWARGS = {}  # test harness may set e.g. {"trace": True}
LAST_RESULTS = None  # BassKernelResults of the last run (for profiling)


def _build():
    nc = bacc.Bacc("TRN2", target_bir_lowering=False, debug=False, num_devices=N_CORES)
    f32, bf16 = mybir.dt.float32, mybir.dt.bfloat16

    kv_d = nc.dram_tensor("kvb", [B, 128, RHALF], bf16, kind="ExternalInput").ap()
    sel_d = nc.dram_tensor("sel", [128, B * MG * 2 * 128], bf16, kind="ExternalInput").ap()
    wt_d = nc.dram_tensor("wt", [128, B * MG], f32, kind="ExternalInput").ap()
    out_d = nc.dram_tensor("out", [B, G, REG], bf16, kind="ExternalOutput").ap()

    with tile.TileContext(nc) as tc:
        with (
            tc.tile_pool(name="rhs", bufs=2) as rhs_pool,
            tc.tile_pool(name="const", bufs=1) as const_pool,
            tc.tile_pool(name="psum", bufs=4, space="PSUM") as psum_pool,
            tc.tile_pool(name="outp", bufs=24) as out_pool,
        ):
            sel_sb = const_pool.tile([128, B * MG * 2 * 128], bf16)
            wt_sb = const_pool.tile([128, B * MG], f32)

            # All input loads issued up front: the batch-0 stripes gate the
            # first matmuls; batch-1 fills DMA-engine idle time during the
            # early compute ramp instead of queueing behind batch-0 stores.
            # The first (small) stripe goes ahead of even sel/wt so the first
            # matmul's data dependency clears as early as possible.
            stripes = [(0, 1024), (1024, 3584), (3584, 6656), (6656, 9728), (9728, RHALF)]
            kv_sbs = [rhs_pool.tile([128, RHALF], bf16, tag="kv", name=f"kv{b}") for b in range(B)]
            # sel/wt on the ACT ring so they load in parallel with batch-0's
            # first stripe on the SP ring — the first matmul needs both, and
            # serializing them on one ring delays all of production ~0.8 us
            nc.scalar.dma_start(sel_sb[:], sel_d)
            nc.scalar.dma_start(wt_sb[:], wt_d)
            # All input up front: batch-0 stripes on the SP ring (no stores
            # exist yet to block), batch-1 stripes alternating across BOTH
            # HWDGE rings so each ring's store stream is frozen only ~half
            # the input time - the 24 out-slots absorb that much production.
            for sidx, (s0, s1) in enumerate(stripes):
                ring = nc.sync if sidx % 2 == 0 else nc.scalar
                ring.dma_start(kv_sbs[0][:, s0:s1], kv_d[0][:, s0:s1])
            for ci in range(8):
                c0_, c1_ = ci * 1568, (ci + 1) * 1568
                ring = nc.sync if ci % 2 == 0 else nc.scalar
                ring.dma_start(kv_sbs[1][:, c0_:c1_], kv_d[1][:, c0_:c1_])

            # PE pstate warmup: 8 junk matmuls on sel data. These are free —
            # the first real matmuls are blocked on input-DMA completion
            # semaphores until ~15 us anyway — and they pre-ramp the PE clock
            # so real matmuls start at ~377 ns/512-col instead of ~630 ns.
            warm_ps = psum_pool.tile([128, 1024], f32, tag="ps", name="warm_ps")
            for w in range(8):
                half = (w % 2) * 512
                nc.tensor.matmul(
                    warm_ps[:, half : half + 512],
                    sel_sb[:, 0:128],
                    sel_sb[:, 256 : 256 + 512],
                    start=True,
                    stop=True,
                )

            ev = 0
            for b in range(B):
                kv_sb = kv_sbs[b]
                for mg in range(MG):
                    wcol = wt_sb[:, b * MG + mg : b * MG + mg + 1]
                    for h in range(2):
                        si = (b * MG + mg) * 2 + h
                        sel_ap = sel_sb[:, si * 128 : (si + 1) * 128]
                        for o0 in range(0, RHALF, OCH):
                            ow = min(OCH, RHALF - o0)
                            ps = psum_pool.tile([128, ow], f32, tag="ps")
                            for c0 in range(0, ow, CH):
                                cw = min(CH, ow - c0)
                                nc.tensor.matmul(
                                    ps[:, c0 : c0 + cw],
                                    sel_ap,
                                    kv_sb[:, o0 + c0 : o0 + c0 + cw],
                                    start=True,
                                    stop=True,
                                )
                            ot = out_pool.tile([128, ow], bf16, tag="ot")
                            dst = out_d[
                                b,
                                mg * 128 : (mg + 1) * 128,
                                h * RHALF + o0 : h * RHALF + o0 + ow,
                            ]
                            # alternate DVE/ACT evictions; each tile's store is
                            # issued from the matching HWDGE ring (SP for DVE
                            # tiles, ACT for its own) to halve per-ring dispatch
                            # pressure
                            if ev % 2 == 0:
                                nc.vector.tensor_scalar_mul(ot[:], ps[:], wcol)
                                nc.sync.dma_start(dst, ot[:])
                            else:
                                nc.scalar.activation(
                                    ot[:],
                                    ps[:],
                                    mybir.ActivationFunctionType.Copy,
                                    scale=wcol,
                                )
                                nc.scalar.dma_start(dst, ot[:])
                            ev += 1

    nc.compile()
    return nc


def _get_nc():
    global _COMPILED
    if _COMPILED is None:
        _COMPILED = _build()
    return _COMPILED


def _prep_core(kv_c: np.ndarray, idx_c: np.ndarray, w_c: np.ndarray) -> dict:
    """kv_c (B, 64, 196, 128) f32, idx_c (B, 64, 4) int, w_c (B, 64, 4) f32."""
    # rhs layout [B, 128, RHALF]: partition h*64 + r = half h of region r (flat)
    kvr = (
        kv_c.reshape(B, P2, 2, RHALF).transpose(0, 2, 1, 3).reshape(B, 128, RHALF)
    )
    kvb = kvr.astype(BF16)

    idx_f = idx_c.reshape(B, G).astype(np.int64)
    w_f = w_c.reshape(B, G).astype(np.float32)

    sel = np.zeros((128, B, MG, 2, 128), dtype=BF16)
    k = np.arange(128)[:, None]
    for b in range(B):
        for mg in range(MG):
            im = idx_f[b, mg * 128 : (mg + 1) * 128][None, :]
            sel[:, b, mg, 0] = (k == im).astype(BF16)
            sel[:, b, mg, 1] = (k == im + 64).astype(BF16)
    sel = sel.reshape(128, B * MG * 2 * 128)

    wt = np.zeros((128, B * MG), dtype=np.float32)
    for b in range(B):
        for mg in range(MG):
            wt[:, b * MG + mg] = w_f[b, mg * 128 : (mg + 1) * 128]

    return {"kvb": kvb, "sel": sel, "wt": wt}


def kernel(r_idx: np.ndarray, r_weight: np.ndarray, kv: np.ndarray) -> np.ndarray:
    global LAST_RESULTS
    nc = _get_nc()
    kv = np.asarray(kv, dtype=np.float32)
    r_idx = np.asarray(r_idx)
    r_weight = np.asarray(r_weight, dtype=np.float32)

    in_maps = [
        _prep_core(
            kv[c * B : (c + 1) * B],
            r_idx[c * B : (c + 1) * B],
            r_weight[c * B : (c + 1) * B],
        )
        for c in range(N_CORES)
    ]

    res = run_bass_kernel_spmd(nc, in_maps, core_ids=list(range(N_CORES)), **RUN_KWARGS)
    LAST_RESULTS = res

    out = np.empty((N, P2, TOPK, W2, C_KV), dtype=np.float32)
    for c in range(N_CORES):
        o = np.asarray(res.results[c]["out"])  # (B, G, REG) bf16
        out[c * B : (c + 1) * B] = o.reshape(B, P2, TOPK, W2, C_KV).astype(np.float32)
    return out
